# revision 6
# baseline (speedup 1.0000x reference)
"""Bidirectional 2-layer LSTM -> dense, Trainium2 Bass kernel, v3.
Like v2 but 2 temporal segments per core (16 total), W=32 warmup.
See kernel_v2.py docstring for the full strategy notes."""

import numpy as np
import ml_dtypes

H = 256
T = 512
D = 128
OUT = 128
FB = 1.0
NSEG = 2          # segments per core
SEG = 32          # real steps per segment
WARM = 32         # warmup steps
N = SEG + WARM    # chain steps per segment
NT = 64           # ZA per-gate column stride (256B, bank-aligned)
LAG = 1
NCORES = 8

_PERM = np.r_[0:256, 512:768, 768:1024, 256:512]
bf16 = ml_dtypes.bfloat16
_CACHE = {}

_WB = {}
_c = 0
_wb_items = [("w0x_f", 1024), ("w0x_b", 1024),
             ("w0h_f", 2048), ("w0h_b", 2048),
             ("w1x_f", 2048), ("w1x_b", 2048),
             ("w1h_f", 2048), ("w1h_b", 2048),
             ("wd_f", 256), ("wd_b", 256)]
for q in range(NSEG):
    _wb_items += [(f"xw_f{q}", N), (f"xw_b{q}", N)]
for _name, _w in _wb_items:
    _WB[_name] = (_c, _c + _w)
    _c += _w
_WBC = _c

_RB = {}
_c = 0
_rb_items = [("b0r_f", 1024), ("b0r_b", 1024), ("ones", 128)]
for q in range(NSEG):
    _rb_items += [(f"b0w_f{q}", 1024), (f"b0w_b{q}", 1024)]
for _name, _w in _rb_items:
    _RB[_name] = (_c, _c + _w)
    _c += _w
_RBC = _c

_F32 = {}
_c = 0
_f32_items = [("ident", 128), ("b1r_f", 8), ("b1r_b", 8)]
for q in range(NSEG):
    _f32_items += [(f"b1w_f{q}", 8), (f"b1w_b{q}", 8),
                   (f"cinitA{q}", 4), (f"cinitB{q}", 4)]
for _name, _w in _f32_items:
    _F32[_name] = (_c, _c + _w)
    _c += _w
_F32C = _c


def _build_program():
    import concourse.mybir as mybir
    from concourse import bacc, tile

    fp32 = mybir.dt.float32
    bft = mybir.dt.bfloat16
    SIGF = mybir.ActivationFunctionType.Sigmoid
    TANHF = mybir.ActivationFunctionType.Tanh
    AO = mybir.AluOpType

    nc = bacc.Bacc(None, target_bir_lowering=False)

    wb_d = nc.declare_dram_parameter("wb", [128, _WBC], bft, isOutput=False)
    rb_d = nc.declare_dram_parameter("rb", [1, _RBC], bft, isOutput=False)
    f32_d = nc.declare_dram_parameter("f32", [128, _F32C], fp32,
                                      isOutput=False)
    hin_d = nc.declare_dram_parameter("hin", [128, 8 * NSEG], bft,
                                      isOutput=False)
    out_d = nc.declare_dram_parameter("out", [2 * SEG * NSEG, OUT], fp32,
                                      isOutput=True)

    with tile.TileContext(nc) as tc:
        with (
            tc.tile_pool(name="pool", bufs=1) as pool,
            tc.tile_pool(name="psum", bufs=1, space="PSUM") as psum,
        ):
            wbt = pool.tile([128, _WBC], bft, tag="wbt")
            rbt = pool.tile([1, _RBC], bft, tag="rbt")
            f32t = pool.tile([128, _F32C], fp32, tag="f32t")
            hint = pool.tile([128, 8 * NSEG], bft, tag="hint")
            outsb = pool.tile([SEG, 2 * NSEG * OUT], fp32, tag="outsb")

            ZA, ZB = {}, {}
            for q in range(NSEG):
                ZA[q] = psum.tile([128, 2 * 8 * NT], fp32, tag=f"ZA{q}", name=f"ZA{q}")
            for q in range(NSEG):
                ZB[q] = psum.tile([128, 512], fp32, tag=f"ZB{q}", name=f"ZB{q}")
            psd = psum.tile([SEG, OUT], fp32, tag="psd")

            st = {}
            for q in range(NSEG):
                for nm, shape, dt_ in [
                        ("HS0", [128, 2 * (N + 1) * 2], bft),
                        ("HS1", [128, 2 * (N + 1) * 2], bft),
                        ("GA", [128, 16], fp32), ("GB", [128, 16], fp32),
                        ("uA", [128, 4], fp32), ("uB", [128, 4], fp32),
                        ("m1A", [128, 4], fp32), ("m1B", [128, 4], fp32),
                        ("cA0", [128, 4], fp32), ("cA1", [128, 4], fp32),
                        ("cB0", [128, 4], fp32), ("cB1", [128, 4], fp32),
                        ("tcA", [128, 4], fp32), ("tcB", [128, 4], fp32)]:
                    st[nm, q] = pool.tile(shape, dt_, tag=f"{nm}_{q}", name=f"{nm}_{q}")

            xw_lo = _WB[f"xw_f0"][0]
            nc.sync.dma_start(wbt[:, xw_lo:_WBC], wb_d[:, xw_lo:_WBC])
            nc.sync.dma_start(wbt[:, 0:2048], wb_d[:, 0:2048])
            nc.sync.dma_start(rbt[:], rb_d[:])
            nc.sync.dma_start(f32t[:], f32_d[:])
            nc.sync.dma_start(hint[:], hin_d[:])
            nc.sync.dma_start(wbt[:, 2048:6144], wb_d[:, 2048:6144])
            nc.sync.dma_start(wbt[:, 6144:xw_lo], wb_d[:, 6144:xw_lo])

            def wbs(name):
                a, b = _WB[name]
                return wbt[:, a:b]

            def rbs(name):
                a, b = _RB[name]
                return rbt[:, a:b]

            def f32s(name):
                a, b = _F32[name]
                return f32t[:, a:b]

            ident = f32s("ident")
            ones = rbs("ones")

            def v22(tl):
                return tl[:].rearrange("p (c k) -> p c k", c=2)

            HS0v = {q: st["HS0", q][:].rearrange("p (c t k) -> p c t k",
                                                 c=2, k=2) for q in range(NSEG)}
            HS1v = {q: st["HS1", q][:].rearrange("p (c t k) -> p c t k",
                                                 c=2, k=2) for q in range(NSEG)}
            ZAv = {q: ZA[q][:].rearrange("p (c g t) -> p c g t", c=2, g=8)
                   for q in range(NSEG)}
            hintv = hint[:].rearrange("p (q x c k) -> p q x c k",
                                      q=NSEG, x=2, c=2)

            for q in range(NSEG):
                nc.vector.tensor_copy(HS0v[q][:, :, 0, :], hintv[:, q, 0])
                nc.vector.tensor_copy(HS1v[q][:, :, 0, :], hintv[:, q, 1])
                nc.vector.tensor_copy(st["cA1", q][:], f32s(f"cinitA{q}"))
                nc.vector.tensor_copy(st["cB1", q][:], f32s(f"cinitB{q}"))

            # ---- preamble: L0 projections into ZA ----
            for q in range(NSEG):
                for ci, dirn in enumerate(("f", "b")):
                    w0x = wbs("w0x_" + dirn)
                    xw = wbs(f"xw_{dirn}{q}")
                    b0w = rbs(f"b0w_{dirn}{q}")
                    b0r = rbs("b0r_" + dirn)
                    for g in range(8):
                        gs = slice(128 * g, 128 * (g + 1))
                        blk = ZAv[q][:, ci, g, 0:N]
                        nc.tensor.matmul(blk, w0x[:, gs], xw,
                                         start=(g == 0), stop=False,
                                         skip_group_check=True)
                        nc.tensor.matmul(blk[:, 0:WARM], b0w[:, gs],
                                         ones[:, 0:WARM], start=False,
                                         stop=False, skip_group_check=True)
                        nc.tensor.matmul(blk[:, WARM:N], b0r[:, gs],
                                         ones[:, 0:SEG], start=False,
                                         stop=False, skip_group_check=True)

            w0h = {ci: wbs("w0h_" + d_) for ci, d_ in enumerate(("f", "b"))}
            w1x = {ci: wbs("w1x_" + d_) for ci, d_ in enumerate(("f", "b"))}
            w1h = {ci: wbs("w1h_" + d_) for ci, d_ in enumerate(("f", "b"))}

            def mv_A(q, t):
                for k in range(2):
                    for ci in range(2):
                        w = w0h[ci]
                        r_ = HS0v[q][:, ci, t, k:k + 1]
                        for g in range(8):
                            col = ZAv[q][:, ci, g, t:t + 1]
                            nc.tensor.matmul(
                                col, w[:, 1024 * k + 128 * g:
                                       1024 * k + 128 * (g + 1)],
                                r_, start=False, stop=(k == 1),
                                skip_group_check=True)

            def mv_B(q, t):
                ZBq = ZB[q]
                for ci, dirn in enumerate(("f", "b")):
                    bt = f32s(f"b1w_{dirn}{q}") if t < WARM \
                        else f32s("b1r_" + dirn)
                    nc.tensor.matmul(ZBq[:, 8 * ci:8 * ci + 8], ident, bt,
                                     start=(ci == 0), stop=False,
                                     skip_group_check=True)
                for k in range(2):
                    for ci in range(2):
                        x_ = HS0v[q][:, ci, t + 1, k:k + 1]
                        wx = w1x[ci]
                        for g in range(8):
                            col = ZBq[:, 8 * ci + g:8 * ci + g + 1]
                            nc.tensor.matmul(
                                col, wx[:, 1024 * k + 128 * g:
                                        1024 * k + 128 * (g + 1)],
                                x_, start=False, stop=False,
                                skip_group_check=True)
                for k in range(2):
                    for ci in range(2):
                        h_ = HS1v[q][:, ci, t, k:k + 1]
                        wh = w1h[ci]
                        for g in range(8):
                            col = ZBq[:, 8 * ci + g:8 * ci + g + 1]
                            nc.tensor.matmul(
                                col, wh[:, 1024 * k + 128 * g:
                                        1024 * k + 128 * (g + 1)],
                                h_, start=False, stop=(k == 1),
                                skip_group_check=True)

            def c_sig(g_out, z_ap):
                nc.scalar.activation(g_out, z_ap, SIGF)

            def c_dve(Gv, u, m1, c_prev, c_cur):
                nc.vector.scalar_tensor_tensor(
                    v22(u), Gv[:, :, 6:8], 0.5, Gv[:, :, 0:2],
                    op0=AO.subtract, op1=AO.mult)
                nc.vector.tensor_mul(v22(m1), Gv[:, :, 2:4], v22(c_prev))
                nc.vector.scalar_tensor_tensor(
                    c_cur[:], u[:], 2.0, m1[:], op0=AO.mult, op1=AO.add)

            def c_tanh(tcx, c_cur):
                nc.scalar.activation(tcx[:], c_cur[:], TANHF)

            def c_h(Gv, tcx, HSv, t):
                tv = v22(tcx)
                nc.vector.tensor_mul(HSv[:, :, t + 1, 0:1],
                                     Gv[:, :, 4:5], tv[:, :, 0:1])
                nc.vector.tensor_mul(HSv[:, :, t + 1, 1:2],
                                     Gv[:, :, 5:6], tv[:, :, 1:2])

            GAv = {q: st["GA", q][:].rearrange("p (c g) -> p c g", c=2)
                   for q in range(NSEG)}
            GBv = {q: st["GB", q][:].rearrange("p (c g) -> p c g", c=2)
                   for q in range(NSEG)}

            for r in range(N + LAG):
                t = r
                tb = r - LAG
                A_on = t < N
                B_on = 0 <= tb < N
                for q in range(NSEG):
                    if A_on:
                        mv_A(q, t)
                    if B_on:
                        mv_B(q, tb)
                for q in range(NSEG):
                    if A_on:
                        c_sig(GAv[q], ZAv[q][:, :, :, t])
                    if B_on:
                        c_sig(st["GB", q][:], ZB[q][:, 0:16])
                for q in range(NSEG):
                    if A_on:
                        cp = st["cA1", q] if t % 2 == 0 else st["cA0", q]
                        cc = st["cA0", q] if t % 2 == 0 else st["cA1", q]
                        c_dve(GAv[q], st["uA", q], st["m1A", q], cp, cc)
                    if B_on:
                        cp = st["cB1", q] if tb % 2 == 0 else st["cB0", q]
                        cc = st["cB0", q] if tb % 2 == 0 else st["cB1", q]
                        c_dve(GBv[q], st["uB", q], st["m1B", q], cp, cc)
                for q in range(NSEG):
                    if A_on:
                        cc = st["cA0", q] if t % 2 == 0 else st["cA1", q]
                        c_tanh(st["tcA", q], cc)
                    if B_on:
                        cc = st["cB0", q] if tb % 2 == 0 else st["cB1", q]
                        c_tanh(st["tcB", q], cc)
                for q in range(NSEG):
                    if A_on:
                        c_h(GAv[q], st["tcA", q], HS0v[q], t)
                    if B_on:
                        c_h(GBv[q], st["tcB", q], HS1v[q], tb)

            # ---- dense ----
            for q in range(NSEG):
                for ci, dirn in enumerate(("f", "b")):
                    wd = wbs("wd_" + dirn)
                    for k in range(2):
                        lhsT = HS1v[q][:, ci, WARM + 1:N + 1, k]
                        nc.tensor.matmul(psd[:], lhsT,
                                         wd[:, 128 * k:128 * (k + 1)],
                                         start=(k == 0), stop=(k == 1),
                                         skip_group_check=True)
                    off = OUT * (2 * q + ci)
                    nc.vector.tensor_copy(outsb[:, off:off + OUT], psd[:])
            for q in range(NSEG):
                for ci in range(2):
                    off = OUT * (2 * q + ci)
                    ro = SEG * (2 * q + ci)
                    nc.sync.dma_start(out_d[ro:ro + SEG, :],
                                      outsb[:, off:off + OUT])

    nc.compile()
    return nc


def _prep_dir_weights(W0, b0, W1, b1):
    W0p = np.ascontiguousarray(W0[:, _PERM], np.float32)
    W1p = np.ascontiguousarray(W1[:512, _PERM], np.float32)
    b0p = b0[_PERM].astype(np.float32).copy()
    b1p = b1[_PERM].astype(np.float32).copy()
    for a in (W0p, W1p):
        a[:, 768:1024] *= 2.0
    for a in (b0p, b1p):
        a[768:1024] *= 2.0
        a[256:512] += FB
    return W0p, b0p, W1p, b1p


def _halves(v):
    return np.stack([v[:128], v[128:]], axis=1).astype(np.float32)


def kernel(x, fw_state, bw_state, Wf0, bf0, Wf1, bf1, Wb0, bb0, Wb1, bb1,
           Wd, bd):
    from concourse.bass_utils import run_bass_kernel_spmd

    x = np.asarray(x, np.float32)
    xr_f = x[-1]
    xr_b = xr_f[::-1]

    Wf0p, bf0p, Wf1p, bf1p = _prep_dir_weights(
        np.asarray(Wf0), np.asarray(bf0), np.asarray(Wf1), np.asarray(bf1))
    Wb0p, bb0p, Wb1p, bb1p = _prep_dir_weights(
        np.asarray(Wb0), np.asarray(bb0), np.asarray(Wb1), np.asarray(bb1))
    Wd = np.asarray(Wd, np.float32)

    forced = np.zeros(1024, np.float32)
    forced[0:256] = -40.0
    forced[256:512] = 40.0
    forced[512:768] = -40.0

    wb_common = np.zeros((128, _WBC), np.float32)

    def put(name, arr):
        a, b = _WB[name]
        wb_common[:, a:b] = arr

    put("w0x_f", Wf0p[0:128])
    put("w0x_b", Wb0p[0:128])
    put("w0h_f", np.concatenate([Wf0p[128:256], Wf0p[256:384]], axis=1))
    put("w0h_b", np.concatenate([Wb0p[128:256], Wb0p[256:384]], axis=1))
    put("w1x_f", np.concatenate([Wf1p[0:128], Wf1p[128:256]], axis=1))
    put("w1x_b", np.concatenate([Wb1p[0:128], Wb1p[128:256]], axis=1))
    put("w1h_f", np.concatenate([Wf1p[256:384], Wf1p[384:512]], axis=1))
    put("w1h_b", np.concatenate([Wb1p[256:384], Wb1p[384:512]], axis=1))
    put("wd_f", np.concatenate([Wd[0:128], Wd[128:256]], axis=1))
    put("wd_b", np.concatenate([Wd[256:384], Wd[384:512]], axis=1))

    fst = np.asarray(fw_state, np.float32)[-1]
    bst = np.asarray(bw_state, np.float32)[-1]

    def b1tile(b1p):
        return b1p.reshape(8, 128).T

    in_maps = []
    for s in range(NCORES):
        wb = wb_common.copy()
        rb = np.zeros((1, _RBC), np.float32)
        rb[0, slice(*_RB["ones"])] = 1.0
        rb[0, slice(*_RB["b0r_f"])] = bf0p
        rb[0, slice(*_RB["b0r_b"])] = bb0p
        f32 = np.zeros((128, _F32C), np.float32)
        f32[:, slice(*_F32["ident"])] = np.eye(128, dtype=np.float32)
        f32[:, slice(*_F32["b1r_f"])] = b1tile(bf1p)
        f32[:, slice(*_F32["b1r_b"])] = b1tile(bb1p)
        hin = np.zeros((128, 8 * NSEG), np.float32)

        for q in range(NSEG):
            seg = s * NSEG + q
            exact = seg == 0
            lo = SEG * seg - WARM
            for name, xr in ((f"xw_f{q}", xr_f), (f"xw_b{q}", xr_b)):
                win = np.zeros((N, D), np.float32)
                for i in range(N):
                    gs = lo + i
                    if gs >= 0:
                        win[i] = xr[gs]
                a, b = _WB[name]
                wb[:, a:b] = win.T
            rb[0, slice(*_RB[f"b0w_f{q}"])] = forced if exact else bf0p
            rb[0, slice(*_RB[f"b0w_b{q}"])] = forced if exact else bb0p
            f32[:, slice(*_F32[f"b1w_f{q}"])] = \
                b1tile(forced if exact else bf1p)
            f32[:, slice(*_F32[f"b1w_b{q}"])] = \
                b1tile(forced if exact else bb1p)
            if exact:
                f32[:, slice(*_F32[f"cinitA{q}"])] = np.concatenate(
                    [_halves(fst[0:256]), _halves(bst[0:256])], axis=1)
                f32[:, slice(*_F32[f"cinitB{q}"])] = np.concatenate(
                    [_halves(fst[512:768]), _halves(bst[512:768])], axis=1)
                hin[:, 8 * q + 0:8 * q + 2] = _halves(fst[256:512])
                hin[:, 8 * q + 2:8 * q + 4] = _halves(bst[256:512])
                hin[:, 8 * q + 4:8 * q + 6] = _halves(fst[768:1024])
                hin[:, 8 * q + 6:8 * q + 8] = _halves(bst[768:1024])

        in_maps.append({
            "wb": wb.astype(bf16),
            "rb": rb.astype(bf16),
            "f32": f32,
            "hin": hin.astype(bf16),
        })

    if "nc" not in _CACHE:
        _CACHE["nc"] = _build_program()
    nc = _CACHE["nc"]

    res = run_bass_kernel_spmd(nc, in_maps, list(range(NCORES)))
    _CACHE["last_result"] = res

    fw_full = np.zeros((T, OUT), np.float32)
    bw_full = np.zeros((T, OUT), np.float32)
    for s in range(NCORES):
        o = np.asarray(res.results[s]["out"])
        for q in range(NSEG):
            seg = s * NSEG + q
            fw_full[SEG * seg:SEG * (seg + 1)] = \
                o[SEG * 2 * q:SEG * (2 * q + 1)]
            bw_full[T - SEG * (seg + 1):T - SEG * seg] = \
                o[SEG * (2 * q + 1):SEG * (2 * q + 2)][::-1]

    logits = fw_full + bw_full + np.asarray(bd, np.float32)[None, :]
    return logits.astype(np.float32)
